# revision 36
# baseline (speedup 1.0000x reference)
"""BigramHash embedding lookup kernel for 8 Trainium2 NeuronCores.

Strategy (row-sharded table, host-side all-to-all since we receive full inputs):
  - Host computes bucket ids h = (prev_id * MULT + id) % NUM_BUCKETS, routes
    each token to the core owning its table shard (SHARD = 250001 rows), and
    sorts by local row id (HBM locality + windowed int16 gather indices).
  - Real tokens are spread evenly over the padded cap slots (pads forward-
    fill the previous id) so position-aligned gather chunks cover the same
    id quantiles on every core — the SPMD program bakes ONE window base per
    chunk, taken as the cross-core min of the chunk's first id (exact,
    data-derived; cache keyed on the bases tuple).
  - Primary path: gpsimd.dma_gather (SWDGE custom op) pulls NT=384 tokens
    per call — the ~1us SWDGE cost is per CALL, so 11 calls replace the 33
    serial indirect DMAs that made the old gather stream ~46us. Indices are
    int16 relative to the chunk's 32768-row window; table rows stored bf16
    padded to 128 cols (256B, the dma_gather granularity).
  - Per 128-token block: PE transpose (64 cols) into a shared [128,128]
    PSUM tile (two blocks -> two PE row groups so their projection matmuls
    stream concurrently), copy to SBUF, 2 matmuls into a 2-bank PSUM tile
    [128,1024] f32, ONE cast to bf16 (alternating vector/scalar engines),
    store. bf16 output halves the dominant HBM write vs f32 (~1e-3 rel err,
    far inside the 2e-2 gate). Host scatters slabs back to token order.
  - Fallback (any window infeasible): per-128-token-block HW indirect DMA
    (int32 ids over the whole shard) — slower but unconditionally correct.
"""

import os as _os
from contextlib import ExitStack

import ml_dtypes
import numpy as np

import concourse.bass as bass
import concourse.mybir as mybir
import concourse.tile as tile
from concourse import bacc, library_config
from concourse.bass import IndirectOffsetOnAxis
from concourse.bass_utils import run_bass_kernel_spmd

VARIANT = _os.environ.get("BIGRAM_VARIANT", "dg")  # "dg" | "ind"
NT = int(_os.environ.get("BIGRAM_NT", "512"))  # max tokens per dma_gather call
# Blocks gathered via resident indirect DMA while the dma_gather ucode
# library loads (~9.6us): the overlay load is kicked explicitly up front
# and these blocks keep the pipeline fed until it completes.
# 0: the SWDGE sem lanes rotate round-robin over all SWDGE DMAs and lock
# to the first queue that uses them, so only queue sequences with period
# dividing 8 are legal — an aperiodic indirect prefix trips the lock.
NPRE = int(_os.environ.get("BIGRAM_NPRE", "0"))

NUM_BUCKETS = 2000003
HASH_DIM = 64
EPAD = 128  # padded row length (bf16) for the 256B dma_gather granularity
MODEL_DIM = 1024
HASH_MULT = 92821
N_CORES = 8
P = 128
SHARD = 250001  # ceil(NUM_BUCKETS / N_CORES); 8*250001 = 2000008 >= NUM_BUCKETS
NFREE = 512  # matmul free dim (one PSUM bank of f32)
W = 32768  # gather window rows (int16 index range)

_prog_cache: dict = {}


def _build_dg_program(K: int, bases: tuple, nts: tuple, modes: tuple) -> "bacc.Bacc":
    """dma_gather path: greedy variable-size chunks, one SWDGE call each.

    Chunks alternate between transpose=True ('t': Q7 desc-gen ~3.8ns/tok but
    2B-granule scatter writes ~9ns/tok of DMA-stream time) and
    transpose=False ('f': Q7 ~6.6ns/tok, contiguous 256B writes ~free) so
    the Q7 generator and the SDMA write path work in parallel instead of
    one idling behind the other."""
    NCH = len(nts)
    assert len(bases) == NCH == len(modes) and sum(nts) == K * P
    nc = bacc.Bacc(
        "TRN2",
        target_bir_lowering=False,
        debug=False,
        num_devices=N_CORES,
        dynamic_dma_scratch_size=65536,
        num_swdge_queues=3,
    )
    f32 = mybir.dt.float32
    bf16 = mybir.dt.bfloat16
    n_pre = sum(1 for m in modes if m == "i")
    ncols = (K * P) // 16
    idx_d = nc.dram_tensor(
        "idx16", [P, ncols], mybir.dt.int16, kind="ExternalInput"
    ).ap()
    idxp_d = None
    if n_pre:
        idxp_d = nc.dram_tensor(
            "idxp", [P, n_pre], mybir.dt.int32, kind="ExternalInput"
        ).ap()
    tab_d = nc.dram_tensor("table", [SHARD, EPAD], bf16, kind="ExternalInput").ap()
    projT_d = nc.dram_tensor(
        "projT", [HASH_DIM, MODEL_DIM], bf16, kind="ExternalInput"
    ).ap()
    ident_d = nc.dram_tensor("ident", [P, P], bf16, kind="ExternalInput").ap()
    out_d = nc.dram_tensor("out", [P * K, MODEL_DIM], bf16, kind="ExternalOutput").ap()

    # block -> (chunk, sub-block) map; chunk col offsets into idx16
    blk_ch, blk_sub, col_off = [], [], []
    off = 0
    for ch, nt in enumerate(nts):
        col_off.append(off)
        off += nt // 16
        for s in range(nt // P):
            blk_ch.append(ch)
            blk_sub.append(s)
    assert len(blk_ch) == K

    with tile.TileContext(nc) as tc, ExitStack() as ctx:
        const_p = ctx.enter_context(tc.tile_pool(name="const", bufs=1))
        idx_p = ctx.enter_context(tc.tile_pool(name="idx", bufs=1))
        # Deep emb pool: with the 3-queue split the gather issues are nearly
        # free, so buffer ALL chunks up front (~1KB/partition each) — at
        # bufs=6 the 7th gather stalled 8.4us waiting for tile recycling.
        emb_p = ctx.enter_context(tc.tile_pool(name="emb", bufs=12))
        embT_p = ctx.enter_context(tc.tile_pool(name="embT", bufs=6))
        out_p = ctx.enter_context(tc.tile_pool(name="out", bufs=8))
        ps_t = ctx.enter_context(tc.tile_pool(name="ps_t", bufs=2, space="PSUM"))
        ps_mm = ctx.enter_context(tc.tile_pool(name="ps_mm", bufs=6, space="PSUM"))

        # Kick the dma_gather ucode overlay load NOW (async ~9.6us); the
        # 'i'-prefix indirect gathers below run on resident firmware in
        # the meantime and don't wait for it.
        if any(m != "i" for m in modes):
            nc.gpsimd.load_library(library_config.mlp)

        # idx first: the gather stream depends only on it. Load the prefix
        # + first chunk's columns separately so gather 0 is ungated fast.
        idx_t = idx_p.tile([P, ncols], mybir.dt.int16)
        idxp_t = None
        if n_pre:
            idxp_t = idx_p.tile([P, n_pre], mybir.dt.int32)
            nc.sync.dma_start(out=idxp_t[:], in_=idxp_d[:])
        f_hi = sum(nts[: n_pre + 1]) // 16
        f_hi = min(f_hi, ncols)
        nc.sync.dma_start(out=idx_t[:, :f_hi], in_=idx_d[:, :f_hi])
        if f_hi < ncols:
            nc.sync.dma_start(out=idx_t[:, f_hi:], in_=idx_d[:, f_hi:])
        ident = const_p.tile([P, P], bf16)
        nc.sync.dma_start(out=ident[:], in_=ident_d[:])
        # rhs rows 64-127: for 't' blocks they multiply the table's zero-pad
        # dims (value irrelevant); for 'f' pairs the duplicate gives block 1
        # its own PE row group.
        projT_s = const_p.tile([P, MODEL_DIM], bf16)
        nc.sync.dma_start(out=projT_s[:HASH_DIM, :], in_=projT_d[:])
        nc.sync.dma_start(out=projT_s[HASH_DIM:, :], in_=projT_d[:])
        # PE warm-up during the DMA-wait ramp: releases the HAM clock gate
        # before the first real matmul.
        dumw = const_p.tile([P, NFREE], bf16)
        nc.vector.memset(dumw[:], 0.0)
        wps = ps_mm.tile([P, NFREE], f32, name="mm")
        for _ in range(8):
            nc.tensor.matmul(
                wps[:], lhsT=dumw[:, :P], rhs=dumw[:], start=True, stop=True
            )

        emb_tiles = {}
        pre_i = 0
        for ch in range(NCH):
            nt = nts[ch]
            if modes[ch] == "i":
                embT = emb_p.tile([P, EPAD], bf16, name="embT")
                nc.gpsimd.indirect_dma_start(
                    out=embT[:],
                    out_offset=None,
                    in_=tab_d[:],
                    in_offset=IndirectOffsetOnAxis(
                        ap=idxp_t[:, pre_i : pre_i + 1], axis=0
                    ),
                )
                pre_i += 1
                emb_tiles[ch] = embT
                continue
            idx_ap = idx_t[:, col_off[ch] : col_off[ch] + nt // 16]
            win = tab_d[bases[ch] : bases[ch] + W, :]
            if modes[ch] == "t":
                embT = emb_p.tile([P, nt], bf16, name="embT")
                emb3 = embT[:].rearrange("p (c t) -> p c t", c=1)
            else:
                embT = emb_p.tile([P, (nt // P) * EPAD], bf16, name="embT")
                emb3 = embT[:].rearrange("p (c e) -> p c e", e=EPAD)
            # 't' and 'f' chunks use separate SWDGE queues so the slow
            # 2B-granule transposed writes don't serialize the fast
            # contiguous 'f' writes behind them in one ring.
            nc.gpsimd.dma_gather(
                emb3, win, idx_ap, nt, nt, EPAD,
                transpose=(modes[ch] == "t"),
                single_packet=False,
                queue_num=1 if modes[ch] == "t" else 2,
            )
            emb_tiles[ch] = embT

        # Processing units: 't' blocks go singly through K=128 matmuls;
        # 'i'/'f' blocks pair (across chunks too) for PE transpose + row
        # groups.
        units = []
        b = 0
        while b < K:
            if modes[blk_ch[b]] == "t":
                units.append([b])
                b += 1
            elif b + 1 < K and modes[blk_ch[b + 1]] != "t":
                units.append([b, b + 1])
                b += 2
            else:
                units.append([b])
                b += 1

        # Blocks 2i/2i+1 share one [P, 2*MODEL_DIM] tile so each PAIR is
        # stored by ONE fused 512KB DMA (3-dim dst AP) — halves the sync
        # queue's issue+wait occupancy and the completion-sem count.
        o2_tiles = {}

        def out_view(bb):
            pid = bb // 2
            if pid not in o2_tiles:
                w = MODEL_DIM if 2 * pid + 1 >= K else 2 * MODEL_DIM
                o2_tiles[pid] = out_p.tile([P, w], bf16, name="o_t")
            return o2_tiles[pid], (bb % 2) * MODEL_DIM

        def maybe_store(bb):
            pid = bb // 2
            if bb != min(2 * pid + 1, K - 1):
                return
            t = o2_tiles[pid]
            s = 2 * pid * P
            if 2 * pid + 1 >= K:
                nc.sync.dma_start(out=out_d[s : s + P, :], in_=t[:])
            else:
                dst = out_d[s : s + 2 * P, :].rearrange("(j p) m -> p j m", j=2)
                nc.sync.dma_start(out=dst, in_=t[:])

        cast_i = 0
        for unit in units:
            ch = blk_ch[unit[0]]
            embT = emb_tiles[ch]
            if modes[ch] == "t":
                (bb,) = unit
                lhsT = embT[:, blk_sub[bb] * P : (blk_sub[bb] + 1) * P]
                o_t, ob = out_view(bb)
                for h in range(MODEL_DIM // NFREE):
                    mm = ps_mm.tile([P, NFREE], f32, name="mm")
                    nc.tensor.matmul(
                        mm[:],
                        lhsT=lhsT,
                        rhs=projT_s[:, h * NFREE : (h + 1) * NFREE],
                        start=True,
                        stop=True,
                    )
                    dst = o_t[:, ob + h * NFREE : ob + (h + 1) * NFREE]
                    if (cast_i + h) % 2 == 0:
                        nc.vector.tensor_copy(dst, mm[:])
                    else:
                        nc.scalar.copy(dst, mm[:])
                maybe_store(bb)
                cast_i += 1
                continue
            nb = len(unit)
            eT_ps = ps_t.tile([nb * HASH_DIM, P], bf16, name="eT_ps")
            for jj, bb in enumerate(unit):
                src = emb_tiles[blk_ch[bb]]
                nc.tensor.transpose(
                    eT_ps[jj * HASH_DIM : (jj + 1) * HASH_DIM, :],
                    src[:, blk_sub[bb] * EPAD : blk_sub[bb] * EPAD + HASH_DIM],
                    ident[:],
                )
            eT = embT_p.tile([nb * HASH_DIM, P], bf16, name="eT")
            if cast_i % 2 == 0:
                nc.vector.tensor_copy(eT[:], eT_ps[:])
            else:
                nc.scalar.copy(eT[:], eT_ps[:])
            views = [out_view(bb) for bb in unit]
            for h in range(MODEL_DIM // NFREE):
                mms = [ps_mm.tile([P, NFREE], f32, name="mm") for _ in range(nb)]
                for jj in range(nb):
                    nc.tensor.matmul(
                        mms[jj][:],
                        lhsT=eT[jj * HASH_DIM : (jj + 1) * HASH_DIM, :],
                        rhs=projT_s[
                            jj * HASH_DIM : (jj + 1) * HASH_DIM,
                            h * NFREE : (h + 1) * NFREE,
                        ],
                        start=True,
                        stop=True,
                    )
                for jj in range(nb):
                    o_t, ob = views[jj]
                    dst = o_t[:, ob + h * NFREE : ob + (h + 1) * NFREE]
                    if (cast_i + jj) % 2 == 0:
                        nc.vector.tensor_copy(dst, mms[jj][:])
                    else:
                        nc.scalar.copy(dst, mms[jj][:])
            for bb in unit:
                maybe_store(bb)
            cast_i += 1
    nc.compile()
    return nc


def _build_ind_program(K: int) -> "bacc.Bacc":
    """Fallback: per-128-token-block indirect DMA gather (int32 ids).

    HW semantics allow only ONE offset per partition per call, so this path
    costs ~1.4us of gpsimd per 128 tokens — correct for any input, slow."""
    nc = bacc.Bacc(
        "TRN2",
        target_bir_lowering=False,
        debug=False,
        num_devices=N_CORES,
        dynamic_dma_scratch_size=65536,
    )
    f32 = mybir.dt.float32
    bf16 = mybir.dt.bfloat16
    idx_d = nc.dram_tensor("idx", [P, K], mybir.dt.int32, kind="ExternalInput").ap()
    tab_d = nc.dram_tensor("table", [SHARD, HASH_DIM], bf16, kind="ExternalInput").ap()
    projT_d = nc.dram_tensor(
        "projT", [HASH_DIM, MODEL_DIM], bf16, kind="ExternalInput"
    ).ap()
    ident_d = nc.dram_tensor("ident", [P, P], bf16, kind="ExternalInput").ap()
    out_d = nc.dram_tensor("out", [P * K, MODEL_DIM], bf16, kind="ExternalOutput").ap()

    with tile.TileContext(nc) as tc, ExitStack() as ctx:
        const_p = ctx.enter_context(tc.tile_pool(name="const", bufs=1))
        idx_p = ctx.enter_context(tc.tile_pool(name="idx", bufs=1))
        emb_p = ctx.enter_context(tc.tile_pool(name="emb", bufs=6))
        embT_p = ctx.enter_context(tc.tile_pool(name="embT", bufs=4))
        out_p = ctx.enter_context(tc.tile_pool(name="out", bufs=6))
        ps_t = ctx.enter_context(tc.tile_pool(name="ps_t", bufs=2, space="PSUM"))
        ps_mm = ctx.enter_context(tc.tile_pool(name="ps_mm", bufs=3, space="PSUM"))

        idx_t = idx_p.tile([P, K], mybir.dt.int32)
        nc.sync.dma_start(out=idx_t[:], in_=idx_d[:])
        ident = const_p.tile([P, P], bf16)
        nc.sync.dma_start(out=ident[:], in_=ident_d[:])
        projT_s = const_p.tile([P, MODEL_DIM], bf16)
        nc.sync.dma_start(out=projT_s[:HASH_DIM, :], in_=projT_d[:])
        nc.sync.dma_start(out=projT_s[HASH_DIM:, :], in_=projT_d[:])

        pbs = list(range(0, K, 2))
        if K % 2:
            pbs = pbs[-1:] + pbs[:-1]
        cast_i = 0
        for pb in pbs:
            nblocks = min(2, K - pb)
            embp = emb_p.tile([P, nblocks * HASH_DIM], bf16)
            for j in range(nblocks):
                nc.gpsimd.indirect_dma_start(
                    out=embp[:, j * HASH_DIM : (j + 1) * HASH_DIM],
                    out_offset=None,
                    in_=tab_d[:],
                    in_offset=IndirectOffsetOnAxis(
                        ap=idx_t[:, pb + j : pb + j + 1], axis=0
                    ),
                )
            eT_ps = ps_t.tile([nblocks * HASH_DIM, P], bf16)
            nc.tensor.transpose(eT_ps[:], embp[:], ident[:])
            eT = embT_p.tile([nblocks * HASH_DIM, P], bf16)
            if cast_i % 2 == 0:
                nc.vector.tensor_copy(eT[:], eT_ps[:])
            else:
                nc.scalar.copy(eT[:], eT_ps[:])
            mms = [ps_mm.tile([P, MODEL_DIM], f32, name="mm") for _ in range(nblocks)]
            for h in range(MODEL_DIM // NFREE):
                for jj in range(nblocks):
                    nc.tensor.matmul(
                        mms[jj][:, h * NFREE : (h + 1) * NFREE],
                        lhsT=eT[jj * HASH_DIM : (jj + 1) * HASH_DIM, :],
                        rhs=projT_s[
                            jj * HASH_DIM : (jj + 1) * HASH_DIM,
                            h * NFREE : (h + 1) * NFREE,
                        ],
                        start=True,
                        stop=True,
                    )
            for jj in range(nblocks):
                o_t = out_p.tile([P, MODEL_DIM], bf16, name="o_t")
                if (cast_i + jj) % 2 == 0:
                    nc.vector.tensor_copy(o_t[:], mms[jj][:])
                else:
                    nc.scalar.copy(o_t[:], mms[jj][:])
                nc.sync.dma_start(
                    out=out_d[(pb + jj) * P : (pb + jj + 1) * P, :], in_=o_t[:]
                )
            cast_i += 1
    nc.compile()
    return nc


def prepare(input_ids, table, proj_w):
    """Route tokens, pick program variant, build per-core in_maps."""
    B, S = input_ids.shape
    T = B * S
    ids = np.asarray(input_ids, dtype=np.int64)
    prev = np.empty_like(ids)
    prev[:, 0] = 0
    prev[:, 1:] = ids[:, :-1]
    h = ((prev * HASH_MULT + ids) % NUM_BUCKETS).reshape(-1)
    owner = h // SHARD
    local = (h - owner * SHARD).astype(np.int64)
    order = np.lexsort((local, owner))
    counts = np.bincount(owner, minlength=N_CORES).astype(np.int64)
    offsets = np.zeros(N_CORES + 1, dtype=np.int64)
    np.cumsum(counts, out=offsets[1:])
    sorted_local = local[order]

    cap = max(P, int(-(-counts.max() // P)) * P)
    K = cap // P

    # Spread each core's real ids evenly over the cap slots; pads forward-
    # fill so the padded sequence stays sorted and chunk windows stay tight.
    pos_list, padded_list = [], []
    for c in range(N_CORES):
        loc = sorted_local[offsets[c] : offsets[c + 1]]
        n = len(loc)
        if n == 0:
            pos = np.zeros(0, dtype=np.int64)
            row = np.zeros(cap, dtype=np.int64)
        else:
            pos = (np.arange(n, dtype=np.int64) * cap) // n
            row = np.zeros(cap, dtype=np.int64)
            row[pos] = loc
            mark = np.full(cap, -1, dtype=np.int64)
            mark[pos] = np.arange(cap, dtype=np.int64)[pos]
            np.maximum.accumulate(mark, out=mark)
            row = row[np.maximum(mark, 0)]
        pos_list.append(pos)
        padded_list.append(row)
    padded_all = np.stack(padded_list)  # [N_CORES, cap]

    # 'i' prefix: NPRE single-block chunks gathered by resident indirect
    # DMA while the dma_gather ucode library loads. Then greedy variable
    # chunks (multiples of 128 tokens, <= NT) with exact cross-core bases.
    n_pre = min(NPRE, K)
    lo_all = padded_all.min(axis=0)
    hi_all = padded_all.max(axis=0)
    nts = [P] * n_pre
    bases = [0] * n_pre
    ok = VARIANT == "dg"
    start = n_pre * P
    while start < cap and ok:
        b = min(max(int(lo_all[start]), 0), SHARD - W)
        limit = min(NT, cap - start)
        size = 0
        for step in range(P, limit + P, P):
            if start + step > cap:
                break
            if int(hi_all[start + step - 1]) - b <= W - 1:
                size = step
            else:
                break
        if size == 0:
            ok = False
            break
        nts.append(size)
        bases.append(b)
        start += size
    if not ok:
        nts, bases, n_pre = [cap], [0], 0
    NCH = len(nts)
    cuts = np.zeros(NCH + 1, dtype=np.int64)
    np.cumsum(nts, out=cuts[1:])
    bases = tuple(bases)
    nts = tuple(nts)
    # Alternate dg modes: 't' (transpose=True) is DMA-write bound, 'f' is
    # Q7 desc-gen bound — interleaving runs both in parallel.
    modes = tuple(
        "i" if ch < n_pre else ("t" if (ch - n_pre) % 2 == 0 else "f")
        for ch in range(NCH)
    )

    table = np.asarray(table, dtype=np.float32)
    projT = np.ascontiguousarray(
        np.asarray(proj_w, dtype=np.float32).T.astype(ml_dtypes.bfloat16)
    )
    in_maps = []
    for c in range(N_CORES):
        lo, hi = c * SHARD, min((c + 1) * SHARD, NUM_BUCKETS)
        ncols_tab = EPAD if ok else HASH_DIM
        shard = np.zeros((SHARD, ncols_tab), dtype=ml_dtypes.bfloat16)
        shard[: hi - lo, :HASH_DIM] = table[lo:hi].astype(ml_dtypes.bfloat16)
        m = {"table": shard, "projT": projT}
        if ok:
            # idx16[p, col_off+s] = chunk token s*16+p, relative to the
            # chunk base; wrapped in 16 partitions, replicated to all 8
            # 16-partition Q7 core groups. 'i' chunks read idxp instead;
            # their idx16 columns are zeroed (int16 can't hold absolutes).
            rel = padded_all[c].copy()
            for ch in range(NCH):
                if modes[ch] == "i":
                    rel[cuts[ch] : cuts[ch + 1]] = 0
                else:
                    rel[cuts[ch] : cuts[ch + 1]] -= bases[ch]
            rel = np.maximum(rel, 0)
            cols = [
                rel[cuts[ch] : cuts[ch + 1]].reshape(-1, 16).T for ch in range(NCH)
            ]
            row16 = np.concatenate(cols, axis=1).astype(np.int16)
            m["idx16"] = np.ascontiguousarray(np.tile(row16, (P // 16, 1)))
            if n_pre:
                m["idxp"] = np.ascontiguousarray(
                    padded_all[c][: n_pre * P]
                    .astype(np.int32)
                    .reshape(n_pre, P)
                    .T
                )
        else:
            padded = np.zeros(cap, dtype=np.int64)
            padded[: counts[c]] = sorted_local[offsets[c] : offsets[c + 1]]
            m["idx"] = np.ascontiguousarray(padded.astype(np.int32).reshape(K, P).T)
        m["ident"] = np.eye(P, dtype=ml_dtypes.bfloat16)
        in_maps.append(m)

    key = ("dg", K, nts, bases, modes) if ok else ("ind", K)
    nc = _prog_cache.get(key)
    if nc is None:
        nc = (
            _build_dg_program(K, bases, nts, modes) if ok else _build_ind_program(K)
        )
        _prog_cache[key] = nc
    # row_map[c]: device out row holding sorted token k of core c
    if ok:
        row_map = pos_list
    else:
        row_map = [np.arange(counts[c], dtype=np.int64) for c in range(N_CORES)]
    meta = (T, order, offsets, row_map, K)
    return nc, in_maps, meta


def kernel(input_ids: np.ndarray, table: np.ndarray, proj_w: np.ndarray) -> np.ndarray:
    B, S = input_ids.shape
    nc, in_maps, meta = prepare(input_ids, table, proj_w)
    T, order, offsets, row_map, K = meta
    res = run_bass_kernel_spmd(nc, in_maps, list(range(N_CORES)))
    flat = np.empty((T, MODEL_DIM), dtype=np.float32)
    for c in range(N_CORES):
        flat[order[offsets[c] : offsets[c + 1]]] = res.results[c]["out"][
            row_map[c]
        ].astype(np.float32)
    return flat.reshape(B, S, MODEL_DIM)
